# revision 5
# baseline (speedup 1.0000x reference)
"""DequantingLinear Trainium2 kernel.

y = x @ W^T + b where W = (w_q - 128) * w_scales (GGML Q8_0-style, block=32),
b = (b_q - 128) * b_scales.

Sharding: column-parallel over out_features across 8 cores (1536 rows of W per
core).  The codes are 8-bit values; the host ships them as int16 (lossless,
like the host-side fp16 cast of x) so the HBM stream is 9.4 MB/core instead of
18.9 MB.  Per core, pipelined per 128-row o-tile:

  1. wq shard streams in contiguously ([128, IN] int16 tiles; first four as
     0.75 MB singles for an early pipeline start, the rest as 1.5 MB pairs)
  2. DVE dequantizes each tile with two fused scalar_tensor_tensor halves:
         wp = (wq - 128) * scales -> fp16
     (f32 scales broadcast along the free dim with a step-0 AP; fp16 W is what
     lets the PE run 1 cycle/row; STT supports no 2x DVE modes so dtype of the
     scales is free)
  3. PE transposes wp 128x128 blocks (is_transpose matmul vs identity) into
     [128, 1024] fp16 PSUM banks laid out as [k-pair][4 o-tiles][128]; ACT
     evacuates each bank in one copy (the matmul needs W^T: contraction must
     be on partitions for both operands, and no AP can swap the partition axis)
  4. PE accumulates y[64, 512] per GROUP of 4 o-tiles in fp32 PSUM: 24 fp16
     matmuls of N=512 (vs N=128 per-tile: 4x fewer matmul instructions) + one
     extra k-tile whose xt rows are a delta at row 0, contracting against a
     bias row tile -> adds the device-dequantized bias
  5. y group slices [64, 512] DMA out as they finish; the host concatenates.

x is transposed/padded on the host (tiny replicated activation); w_scales are
host-prearranged into the exact SBUF tile layout so every const DMA is fully
contiguous (the baseline's strided const loads wasted ~8 us of ramp at
100-190 GB/s).

Two TRN2 toolchain quirks are handled explicitly (see _strip_self_waits and
_patch_drain_split): every ISA instruction encodes at most ONE semaphore
wait, and walrus refuses multi-wait encodings for several instruction
structs ("Too many sync wait commands").  Cheap same-engine "absorber" ops
take the DMA/slot-release waits up front, a post-pass drops provably
redundant waits (self-engine ordering; DMA-lane waits transitively covered
by consumer-engine waits), and the kernel-tail drain's global-clock waits
are pre-spread across SP nops.
"""

import sys

import numpy as np

for _p in ("/opt/trn_rl_repo", "/root/.axon_site/_ro/trn_rl_repo"):
    if _p not in sys.path:
        sys.path.append(_p)

B = 64          # batch (x is [64, 1, 3072])
IN = 3072       # in_features
OUT = 12288     # out_features
BLOCK = 32      # quant block
NB = IN // BLOCK            # 96 blocks per row
NCORES = 8
OSH = OUT // NCORES         # 1536 out features per core
OT = OSH // 128             # 12 o-tiles of 128 rows per core
GRP = 4                     # o-tiles per matmul group (N = 512)
NG = OT // GRP              # 3 groups
KT = IN // 128              # 24 contraction tiles
NSING = 4                   # leading wq tiles loaded as singles

_CACHE: dict = {}


def _patch_drain_split():
    """The TRN2 ISA gives every instruction exactly ONE inline wait slot;
    Tile's kernel-tail drain asks for the whole global clock (~11 sems) on a
    single instruction, which walrus sometimes refuses ("Too many sync wait
    commands").  Pre-spread those waits across one SP nop per semaphore; the
    drain's own waits then elide via the SP engine clock."""
    from concourse import tile as tile_mod

    if getattr(tile_mod.TileContext, "_drain_split_patched", False):
        return
    from concourse.vector_clock import ScopedClock, VectorClock

    orig = tile_mod.TileContext._drain_and_barrier

    def patched(self, tick_clock, wait_clock):
        gvc = tick_clock.global_clock
        n = len(gvc)
        for p in range(n):
            t = gvc[p]
            if t <= 0:
                continue
            vc = VectorClock([0] * n)
            vc.require_at_least(p, t)
            nop = self.nc.sync.nop(hint="drain_wait_split", nofuse=True)
            wait_clock.add_sem_waits(nop.ins, ScopedClock({None: vc}))
        return orig(self, tick_clock, wait_clock)

    tile_mod.TileContext._drain_and_barrier = patched
    tile_mod.TileContext._drain_split_patched = True


def _build_nc():
    import concourse.bass as bass
    import concourse.mybir as mybir
    from concourse.tile import TileContext
    from contextlib import ExitStack

    _patch_drain_split()

    f32 = mybir.dt.float32
    i16 = mybir.dt.int16
    f16 = mybir.dt.float16

    nc = bass.Bass()
    wq = nc.declare_dram_parameter("wq", [OSH, IN], i16, isOutput=False)
    # ws is host-prearranged: ws[p, t*NB+k] = w_scales[t*128+p, k] so the load
    # is one fully contiguous [128, OT*NB] transfer.
    ws = nc.declare_dram_parameter("ws", [128, OT * NB], f32, isOutput=False)
    # xt is host-prearranged: xt[p, n*B+b] = x^T[n*128+p, b] for n < KT, and
    # the last B columns are a delta at row 0 (bias ones-row k-tile).
    xt = nc.declare_dram_parameter("xt", [128, (KT + 1) * B], f16, isOutput=False)
    bq = nc.declare_dram_parameter("bq", [1, OSH], i16, isOutput=False)
    bs = nc.declare_dram_parameter("bs", [1, OSH // BLOCK], f32, isOutput=False)
    ident = nc.declare_dram_parameter("ident", [128, 128], f16, isOutput=False)
    y = nc.declare_dram_parameter("y", [B, OSH], f32, isOutput=True)

    with TileContext(nc) as tc, ExitStack() as ctx:
        const = ctx.enter_context(tc.tile_pool(name="const", bufs=1))
        wq1_pool = ctx.enter_context(tc.tile_pool(name="wq1", bufs=NSING))
        wq_pool = ctx.enter_context(tc.tile_pool(name="wq", bufs=3))
        wp_pool = ctx.enter_context(tc.tile_pool(name="wp", bufs=6))
        wpt_pool = ctx.enter_context(tc.tile_pool(name="wpt", bufs=4))
        ysb_pool = ctx.enter_context(tc.tile_pool(name="ysb", bufs=1))
        pt_pool = ctx.enter_context(tc.tile_pool(name="pt", bufs=3, space="PSUM"))  # [128,1024] fp16 = 1 bank each
        py_pool = ctx.enter_context(tc.tile_pool(name="py", bufs=2, space="PSUM"))
        scrap_pool = ctx.enter_context(tc.tile_pool(name="scrap", bufs=1, space="PSUM"))

        # --- constants / small inputs (all fully contiguous transfers) ---
        s_all = const.tile([128, OT * NB], f32)
        nc.sync.dma_start(s_all[:], ws[:, :])
        xt_sb = const.tile([128, (KT + 1) * B], f16)
        nc.sync.dma_start(xt_sb[:], xt[:, :])
        id_sb = const.tile([128, 128], f16)
        nc.sync.dma_start(id_sb[:], ident[:, :])

        # wq DMAs issued right after the consts so the big stream starts as
        # early as possible; first NSING tiles as singles (earlier per-tile
        # completion), the rest as pairs (fewer descgen instructions on SP).
        wq_first = []
        for t in range(NSING):
            wq_s = wq1_pool.tile([128, IN], i16)
            nc.sync.dma_start(wq_s[:], wq[128 * t : 128 * (t + 1), :])
            wq_first.append(wq_s)
        wq_pair = []
        for h in range(NSING // 2, OT // 2):
            wq_t = wq_pool.tile([128, 2 * IN], i16)
            nc.sync.dma_start(
                wq_t[:].rearrange("p (t f) -> p t f", t=2),
                wq[256 * h : 256 * (h + 1), :].rearrange(
                    "(t p) f -> p t f", p=128
                ),
            )
            wq_pair.append(wq_t)

        scr = const.tile([1, 64], f32)
        bq_sb = const.tile([1, OSH], i16)
        nc.sync.dma_start(bq_sb[:], bq[:, :])
        bs_sb = const.tile([1, OSH // BLOCK], f32)
        nc.sync.dma_start(bs_sb[:], bs[:, :])

        # Wait-absorber: one cheap DVE op takes the s_all DMA wait so the
        # first dequant STT carries only its wq DMA wait (one-wait ISA limit).
        nc.vector.tensor_copy(scr[0:1, 3:4], s_all[0:1, 0:1])

        # bias dequant (single partition, 1536 elems — overlaps the ramp).
        # bias_sb is fp16: it only feeds the fp16 bias-row matmul tile.
        bias_sb = const.tile([1, OSH], f16)
        nc.vector.tensor_copy(scr[0:1, 0:1], bq_sb[0:1, 0:1])
        nc.vector.tensor_copy(scr[0:1, 1:2], bs_sb[0:1, 0:1])
        nc.vector.scalar_tensor_tensor(
            bias_sb[:].rearrange("o (k j) -> o k j", j=BLOCK),
            bq_sb[:].rearrange("o (k j) -> o k j", j=BLOCK),
            128.0,
            bs_sb[:].unsqueeze(2).broadcast_to([1, OSH // BLOCK, BLOCK]),
            mybir.AluOpType.subtract,
            mybir.AluOpType.mult,
        )
        # bias row tile for all groups: row 0 = bias, rows 1..127 = 0;
        # contracted against the delta k-tile of xt.  Built once on DVE (one
        # writer sem: the bias matmul's ISA struct holds a single wait),
        # during the ramp while DVE is otherwise idle.
        wpt_x = const.tile([128, OSH], f16)
        nc.vector.memset(wpt_x[:], 0.0)
        nc.vector.tensor_copy(wpt_x[0:1, :], bias_sb[0:1, :])

        y_sb = ysb_pool.tile([B, OSH], f32)

        # PE wait-absorbers: the matmul LW ISA struct carries at most one
        # sync wait.  Touch each constant input with a K=128 M=1 N=1 matmul so
        # the one-time DMA waits are spread over separate PE instructions.
        scrap = scrap_pool.tile([1, 4], f32)
        for i, src in enumerate((id_sb, xt_sb)):
            nc.tensor.matmul(
                scrap[0:1, i : i + 1], src[:, 0:1], src[:, 0:1],
                start=True, stop=True,
            )

        wp = [None] * OT

        def dequant(t):
            if t < NSING:
                wq_t = wq_first[t][:, :]
            else:
                wq_t = wq_pair[t // 2 - NSING // 2][
                    :, IN * (t % 2) : IN * (t % 2 + 1)
                ]
            wp_t = wp_pool.tile([128, IN], f16)
            wp[t] = wp_t
            # absorber: memset takes the wp slot-release wait; the STT then
            # carries only the wq DMA-completion wait.
            nc.vector.memset(wp_t[0:1, 0:1], 0.0)
            for hh in range(2):
                sl = slice(hh * IN // 2, (hh + 1) * IN // 2)
                nc.vector.scalar_tensor_tensor(
                    wp_t[:, sl].rearrange("p (k j) -> p k j", j=BLOCK),
                    wq_t[:, sl].rearrange("p (k j) -> p k j", j=BLOCK),
                    128.0,
                    s_all[:, t * NB + hh * NB // 2 : t * NB + (hh + 1) * NB // 2]
                    .unsqueeze(2)
                    .broadcast_to([128, NB // 2, BLOCK]),
                    mybir.AluOpType.subtract,
                    mybir.AluOpType.mult,
                )

        for g in range(NG):
            for t in range(GRP * g, GRP * (g + 1)):
                dequant(t)
            py = py_pool.tile([B, GRP * 128], f32)
            # 2 k-slices x 4 o-tiles per [128, 1024] fp16 psum bank:
            # 8 transposes then ONE ACT evacuation, then two N=512 matmuls.
            for kb in range(KT // 2):
                pt = pt_pool.tile([128, 1024], f16)
                for j in range(8):
                    k = 2 * kb + j // GRP
                    t = GRP * g + j % GRP
                    nc.tensor.transpose(
                        pt[:, 128 * j : 128 * (j + 1)],
                        wp[t][:, 128 * k : 128 * (k + 1)],
                        id_sb[:],
                    )
                wpt = wpt_pool.tile([128, 1024], f16)
                nc.scalar.copy(wpt[:], pt[:])
                for kk in range(2):
                    k = 2 * kb + kk
                    nc.tensor.matmul(
                        py[:],
                        xt_sb[:, B * k : B * (k + 1)],
                        wpt[:, 512 * kk : 512 * (kk + 1)],
                        start=(k == 0),
                        stop=False,
                    )
            # += bias via the delta k-tile (K=128 like every other matmul)
            nc.tensor.matmul(
                py[:],
                xt_sb[:, B * KT : B * (KT + 1)],
                wpt_x[:, 512 * g : 512 * (g + 1)],
                start=False,
                stop=True,
            )
            nc.scalar.copy(y_sb[:, 512 * g : 512 * (g + 1)], py[:])
            nc.sync.dma_start(
                y[:, 512 * g : 512 * (g + 1)],
                y_sb[:, 512 * g : 512 * (g + 1)],
            )

    _strip_self_waits(nc, mybir)
    return nc


# NOTE: Pool (GPSIMD) is deliberately absent — it is 8 parallel Q7 cores, so
# same-engine ordering does NOT hold there and its self-waits are load-bearing.
_ENGINE_SEM_PREFIX = {
    "PE": "PE_",
    "DVE": "DVE_",
    "Activation": "Activation_",
    "SP": "SP_",
}


def _strip_self_waits(nc, mybir):
    """Several TRN2 ISA instruction structs encode at most ONE sync wait
    (walrus: "Too many sync wait commands").  Two classes of Tile-emitted
    waits are redundant and safe to drop from instructions carrying >=2:

    1. Self-engine waits: an engine completes its own instructions in order.
    2. DMAHW waits on the wq streaming loads: the slot's previous DMA was
       fully consumed by the DVE dequant before the slot-release (DVE) wait
       tick, so the DVE wait transitively covers the DMA-WAW ordering (Tile's
       per-proc vector clock does not track transitivity).
    """
    fn = nc.m.functions[0]
    # (engine, sem) -> highest value this engine has already waited for.  An
    # engine's instruction stream executes in order through the linear block
    # chain, so any later wait with value <= that is redundant.
    observed: dict = {}
    for b in fn.blocks:
        for inst in b.instructions:
            si = inst.sync_info
            if si is None or not si.on_wait:
                continue
            eng = str(inst.engine)
            if len(si.on_wait) < 2:
                for w in si.on_wait:
                    k = (eng, w.ant_name)
                    observed[k] = max(observed.get(k, 0), w.wait_value)
                continue
            keep = [
                w
                for w in si.on_wait
                if observed.get((eng, w.ant_name), 0) < w.wait_value
            ]
            pref = _ENGINE_SEM_PREFIX.get(str(inst.engine).split(".")[-1])
            if pref is not None:
                keep = [w for w in keep if not w.ant_name.startswith(pref)]
            if len(keep) >= 2 and type(inst).__name__ == "InstDMACopy":
                # In this kernel every DMA's cross-lane (DMAHW) waits guard
                # slot reuse whose previous reader/writer chain ends in the
                # compute-engine wait Tile also emitted — transitively
                # covered, so keep only the engine-sem wait.
                if any(
                    not w.ant_name.startswith(("DMAHW", "DMASW")) for w in keep
                ):
                    keep = [
                        w
                        for w in keep
                        if not w.ant_name.startswith(("DMAHW", "DMASW"))
                    ]
            for w in keep:
                k = (eng, w.ant_name)
                observed[k] = max(observed.get(k, 0), w.wait_value)
            if len(keep) != len(si.on_wait):
                inst.sync_info = mybir.SyncInfo(
                    on_wait=keep, on_update=si.on_update
                )


def _get_nc():
    if "nc" not in _CACHE:
        _CACHE["nc"] = _build_nc()
    return _CACHE["nc"]


def _make_in_maps(x, w_q, w_scales, b_q, b_scales):
    x2 = np.ascontiguousarray(x.reshape(B, IN), dtype=np.float32)
    xT = x2.T.astype(np.float16)                          # [3072, 64]
    xt = np.zeros((128, (KT + 1) * B), dtype=np.float16)  # [128, 1600]
    xt[:, : KT * B] = (
        xT.reshape(KT, 128, B).transpose(1, 0, 2).reshape(128, KT * B)
    )
    xt[0, KT * B :] = 1.0                                 # bias delta row
    # int32 codes are 8-bit values: ship as int16 (lossless; halves the
    # dominant HBM stream the same way x ships as fp16).
    wq_full = np.ascontiguousarray(
        w_q.reshape(OUT, IN), dtype=np.int16
    )
    ws_full = np.ascontiguousarray(w_scales, dtype=np.float32)  # [12288, 96]
    bq_full = np.ascontiguousarray(b_q.reshape(OUT), dtype=np.int16)
    bs_full = np.ascontiguousarray(b_scales, dtype=np.float32)  # [384]
    ident = np.eye(128, dtype=np.float16)

    in_maps = []
    for c in range(NCORES):
        o0, o1 = c * OSH, (c + 1) * OSH
        ws_c = (
            ws_full[o0:o1]
            .reshape(OT, 128, NB)
            .transpose(1, 0, 2)
            .reshape(128, OT * NB)
        )
        in_maps.append(
            {
                "wq": np.ascontiguousarray(wq_full[o0:o1]),
                "ws": np.ascontiguousarray(ws_c),
                "xt": xt,
                "bq": np.ascontiguousarray(bq_full[o0:o1]).reshape(1, OSH),
                "bs": np.ascontiguousarray(
                    bs_full[o0 // BLOCK : o1 // BLOCK]
                ).reshape(1, OSH // BLOCK),
                "ident": ident,
            }
        )
    return in_maps


def run_shards(x, w_q, w_scales, b_q, b_scales, trace=False):
    """Run the SPMD kernel; returns (y_full, BassKernelResults)."""
    from concourse.bass_utils import run_bass_kernel_spmd

    nc = _get_nc()
    in_maps = _make_in_maps(x, w_q, w_scales, b_q, b_scales)
    res = run_bass_kernel_spmd(
        nc, in_maps, core_ids=list(range(NCORES)), trace=trace
    )
    shards = [np.asarray(res.results[c]["y"]) for c in range(NCORES)]
    y = np.concatenate(shards, axis=1).reshape(B, 1, OUT)
    return y, res


def kernel(**inputs):
    y, _ = run_shards(
        inputs["x"],
        inputs["w_q"],
        inputs["w_scales"],
        inputs["b_q"],
        inputs["b_scales"],
        trace=False,
    )
    return y.astype(np.float32)


# revision 8
# speedup vs baseline: 1.0516x; 1.0516x over previous
"""DequantingLinear Trainium2 kernel, v3.

y = x @ W^T + b where W = (w_q - 128) * w_scales (GGML Q8_0-style, block=32),
b = (b_q - 128) * b_scales.

Sharding: column-parallel over out_features across 8 cores (1536 rows of W per
core).  The codes are 8-bit values; the host ships them as int16 (lossless,
like the host-side fp16 cast of x) so the HBM stream is 9.4 MB/core.

Per core, pipelined per 128-row o-tile with the engines balanced so no single
engine exceeds the others (v2 learning: DVE dequant 44 us, ACT evac 42 us and
PE 51 us all serialized behind group-granular dependencies):

  1. wq shard streams in ([128, IN] int16 tiles, first four as singles)
  2. dequant wp = (wq - 128) * scales -> fp16, fused scalar_tensor_tensor:
     DVE for 9 tiles, GPSIMD (Pool) for the first tile of each group (Pool is
     otherwise idle; per-tile assignment keeps each consumer waiting on ONE
     producer semaphore).  The bias dequant chain lives entirely on Pool too.
  3. PE transposes wp *as fp32 pairs*: a [128, 128] f32 transpose moves two
     adjacent-i fp16 values per element, so 12 transposes per o-tile instead
     of 24 (the per-instruction LDWEIGHTS + sem-wait overhead halves), into
     [128, 512] f32 PSUM banks (4 i-blocks of one tile per bank).  Packed
     pairs are safe: |W| < 3 so the composed f32 exponent field can never be
     all-ones (no NaN/Inf), and a denormal high-half implies the whole block
     scale is tiny so flush-to-zero error is negligible.
  4. ACT evacuates each bank as f32 (half the elements of the fp16 view) into
     a per-group [128, 12x4x128] f32 staging tile.
  5. PE accumulates y[64, 512] per group of 4 o-tiles in fp32 PSUM: for each
     i-block bb and parity e, one N=512 fp16 matmul whose rhs reads the fp16
     view of the staging tile with stride-2 APs; xt is host-interleaved to
     match (partition p of block (bb,e) holds x^T row 256*bb + 2p + e).
     One extra delta-row k-tile adds the Pool-dequantized bias.
  6. y group slices [64, 512] DMA out as they finish; the host concatenates.

Two TRN2 toolchain quirks are handled explicitly (see _strip_self_waits and
_patch_drain_split): every ISA instruction encodes at most ONE semaphore
wait, and walrus refuses multi-wait encodings for several instruction
structs ("Too many sync wait commands").  Cheap same-engine "absorber" ops
take the DMA/slot-release waits up front, a post-pass drops provably
redundant waits, and the kernel-tail drain's global-clock waits are
pre-spread across SP nops.
"""

import sys

import numpy as np

for _p in ("/opt/trn_rl_repo", "/root/.axon_site/_ro/trn_rl_repo"):
    if _p not in sys.path:
        sys.path.append(_p)

B = 64          # batch (x is [64, 1, 3072])
IN = 3072       # in_features
OUT = 12288     # out_features
BLOCK = 32      # quant block
NB = IN // BLOCK            # 96 blocks per row
NCORES = 8
OSH = OUT // NCORES         # 1536 out features per core
OT = OSH // 128             # 12 o-tiles of 128 rows per core
GRP = 4                     # o-tiles per matmul group (N = 512)
NG = OT // GRP              # 3 groups
KT = IN // 128              # 24 contraction tiles
NBB = IN // 256             # 12 fp32-pair i-blocks per o-tile
NSING = 4                   # leading wq tiles loaded as singles

# o-tiles whose dequant runs on GPSIMD (Pool) instead of DVE.  First tile of
# each group: its (slower) dequant overlaps the previous group's compute and
# never gates the group's matmul accumulation tail.
POOL_TILES = frozenset((0, 4, 8))

_CACHE: dict = {}


def _patch_drain_split():
    """The TRN2 ISA gives every instruction exactly ONE inline wait slot;
    Tile's kernel-tail drain asks for the whole global clock (~11 sems) on a
    single instruction, which walrus sometimes refuses ("Too many sync wait
    commands").  Pre-spread those waits across one SP nop per semaphore; the
    drain's own waits then elide via the SP engine clock."""
    from concourse import tile as tile_mod

    if getattr(tile_mod.TileContext, "_drain_split_patched", False):
        return
    from concourse.vector_clock import ScopedClock, VectorClock

    orig = tile_mod.TileContext._drain_and_barrier

    def patched(self, tick_clock, wait_clock):
        gvc = tick_clock.global_clock
        n = len(gvc)
        for p in range(n):
            t = gvc[p]
            if t <= 0:
                continue
            vc = VectorClock([0] * n)
            vc.require_at_least(p, t)
            nop = self.nc.sync.nop(hint="drain_wait_split", nofuse=True)
            wait_clock.add_sem_waits(nop.ins, ScopedClock({None: vc}))
        return orig(self, tick_clock, wait_clock)

    tile_mod.TileContext._drain_and_barrier = patched
    tile_mod.TileContext._drain_split_patched = True


def _build_nc():
    import concourse.bass as bass
    import concourse.mybir as mybir
    from concourse.tile import TileContext
    from contextlib import ExitStack

    _patch_drain_split()

    f32 = mybir.dt.float32
    i16 = mybir.dt.int16
    f16 = mybir.dt.float16

    nc = bass.Bass()
    wq = nc.declare_dram_parameter("wq", [OSH, IN], i16, isOutput=False)
    # ws is host-prearranged: ws[p, t*NB+k] = w_scales[t*128+p, k] so the load
    # is one fully contiguous [128, OT*NB] transfer.
    ws = nc.declare_dram_parameter("ws", [128, OT * NB], f32, isOutput=False)
    # xt is host-prearranged and parity-interleaved: for block bb and parity
    # e, column (2*bb+e)*B + b holds x^T[256*bb + 2*p + e, b] at partition p.
    # The last B columns are a delta at row 0 (bias k-tile).
    xt = nc.declare_dram_parameter("xt", [128, (KT + 1) * B], f16, isOutput=False)
    bq = nc.declare_dram_parameter("bq", [1, OSH], i16, isOutput=False)
    bs = nc.declare_dram_parameter("bs", [1, OSH // BLOCK], f32, isOutput=False)
    ident = nc.declare_dram_parameter("ident", [128, 128], f32, isOutput=False)
    y = nc.declare_dram_parameter("y", [B, OSH], f32, isOutput=True)

    with TileContext(nc) as tc, ExitStack() as ctx:
        const = ctx.enter_context(tc.tile_pool(name="const", bufs=1))
        wq1_pool = ctx.enter_context(tc.tile_pool(name="wq1", bufs=NSING))
        wq_pool = ctx.enter_context(tc.tile_pool(name="wq", bufs=3))
        wp_pool = ctx.enter_context(tc.tile_pool(name="wp", bufs=6))
        wg_pool = ctx.enter_context(tc.tile_pool(name="wg", bufs=2))
        ysb_pool = ctx.enter_context(tc.tile_pool(name="ysb", bufs=1))
        pt_pool = ctx.enter_context(tc.tile_pool(name="pt", bufs=4, space="PSUM"))  # [128,512] f32 = 1 bank each
        py_pool = ctx.enter_context(tc.tile_pool(name="py", bufs=2, space="PSUM"))
        scrap_pool = ctx.enter_context(tc.tile_pool(name="scrap", bufs=1, space="PSUM"))

        # --- constants / small inputs (all fully contiguous transfers) ---
        s_all = const.tile([128, OT * NB], f32)
        nc.sync.dma_start(s_all[:], ws[:, :])
        xt_sb = const.tile([128, (KT + 1) * B], f16)
        nc.sync.dma_start(xt_sb[:], xt[:, :])
        id_sb = const.tile([128, 128], f32)
        nc.sync.dma_start(id_sb[:], ident[:, :])

        # wq DMAs issued right after the consts so the big stream starts as
        # early as possible; first NSING tiles as singles (earlier per-tile
        # completion), the rest as pairs (fewer descgen instructions on SP).
        wq_first = []
        for t in range(NSING):
            wq_s = wq1_pool.tile([128, IN], i16)
            nc.sync.dma_start(wq_s[:], wq[128 * t : 128 * (t + 1), :])
            wq_first.append(wq_s)
        wq_pair = []
        for h in range(NSING // 2, OT // 2):
            wq_t = wq_pool.tile([128, 2 * IN], i16)
            nc.sync.dma_start(
                wq_t[:].rearrange("p (t f) -> p t f", t=2),
                wq[256 * h : 256 * (h + 1), :].rearrange(
                    "(t p) f -> p t f", p=128
                ),
            )
            wq_pair.append(wq_t)

        scr = const.tile([1, 64], f32)
        scr2 = const.tile([1, 64], f32)
        bq_sb = const.tile([1, OSH], i16)
        nc.sync.dma_start(bq_sb[:], bq[:, :])
        bs_sb = const.tile([1, OSH // BLOCK], f32)
        nc.sync.dma_start(bs_sb[:], bs[:, :])

        # Wait-absorbers: one cheap op per consumer engine takes the s_all
        # DMA wait so the first dequant STT on each engine carries only its
        # wq DMA wait (one-wait ISA limit).
        nc.vector.tensor_copy(scr[0:1, 3:4], s_all[0:1, 0:1])
        nc.gpsimd.tensor_copy(scr2[0:1, 3:4], s_all[0:1, 0:1])

        # bias chain entirely on Pool (single producer semaphore for the bias
        # matmul): dequant bias (codes are host-pre-centered, so dequant is a
        # plain multiply — TensorTensor is the op GPSIMD supports), build the
        # bias row tile (row 0 = bias, rows 1..127 = 0).
        bias_sb = const.tile([1, OSH], f16)
        nc.gpsimd.tensor_copy(scr2[0:1, 0:1], bq_sb[0:1, 0:1])
        nc.gpsimd.tensor_copy(scr2[0:1, 1:2], bs_sb[0:1, 0:1])
        nc.gpsimd.tensor_tensor(
            bias_sb[:].rearrange("o (k j) -> o k j", j=BLOCK),
            bq_sb[:].rearrange("o (k j) -> o k j", j=BLOCK),
            bs_sb[:].unsqueeze(2).broadcast_to([1, OSH // BLOCK, BLOCK]),
            mybir.AluOpType.mult,
        )
        wpt_x = const.tile([128, OSH], f16)
        nc.gpsimd.memset(wpt_x[:], 0.0)
        nc.gpsimd.tensor_copy(wpt_x[0:1, :], bias_sb[0:1, :])

        y_sb = ysb_pool.tile([B, OSH], f32)

        # PE wait-absorbers: the matmul LW ISA struct carries at most one
        # sync wait.  Touch each constant input with a K=128 M=1 N=1 matmul so
        # the one-time DMA waits are spread over separate PE instructions.
        scrap = scrap_pool.tile([1, 4], f32)
        for i, src in enumerate((id_sb, xt_sb)):
            nc.tensor.matmul(
                scrap[0:1, i : i + 1], src[:, 0:1], src[:, 0:1],
                start=True, stop=True,
            )

        wp = [None] * OT

        def dequant(t):
            if t < NSING:
                wq_t = wq_first[t][:, :]
            else:
                wq_t = wq_pair[t // 2 - NSING // 2][
                    :, IN * (t % 2) : IN * (t % 2 + 1)
                ]
            eng = nc.gpsimd if t in POOL_TILES else nc.vector
            wp_t = wp_pool.tile([128, IN], f16)
            wp[t] = wp_t
            # absorber: memset takes the wp slot-release wait; the dequant
            # multiply then carries only the wq DMA-completion wait.  Codes
            # are host-pre-centered (q - 128), so dequant is a single
            # TensorTensor multiply, which both DVE and GPSIMD support.
            eng.memset(wp_t[0:1, 0:1], 0.0)
            for hh in range(2):
                sl = slice(hh * IN // 2, (hh + 1) * IN // 2)
                eng.tensor_tensor(
                    wp_t[:, sl].rearrange("p (k j) -> p k j", j=BLOCK),
                    wq_t[:, sl].rearrange("p (k j) -> p k j", j=BLOCK),
                    s_all[:, t * NB + hh * NB // 2 : t * NB + (hh + 1) * NB // 2]
                    .unsqueeze(2)
                    .broadcast_to([128, NB // 2, BLOCK]),
                    mybir.AluOpType.mult,
                )

        for g in range(NG):
            for t in range(GRP * g, GRP * (g + 1)):
                dequant(t)
            py = py_pool.tile([B, GRP * 128], f32)
            # Per-group staging tile for transposed W: f32 layout
            # [p, (bb, t_loc, o)] — evacs write per (tile, 4-block) bank,
            # matmuls read per (bb, e) with a stride-2 fp16 view.
            wpt_g = wg_pool.tile([128, NBB * GRP * 128], f32)
            wpt_g16 = wpt_g[:].bitcast(f16)
            mm = 0
            for q in range(NBB // GRP):          # 3 pt banks per o-tile
                for j in range(GRP):
                    t = GRP * g + j
                    wp32 = wp[t][:].bitcast(f32)  # [128, 1536]
                    pt = pt_pool.tile([128, 512], f32)
                    for bi in range(GRP):
                        bb = GRP * q + bi
                        nc.tensor.transpose(
                            pt[:, 128 * bi : 128 * (bi + 1)],
                            wp32[:, 128 * bb : 128 * (bb + 1)],
                            id_sb[:],
                        )
                    nc.scalar.copy(
                        wpt_g[:]
                        .rearrange("p (bb t o) -> p bb t o", t=GRP, o=128)[
                            :, GRP * q : GRP * (q + 1), j, :
                        ],
                        pt[:].rearrange("p (bb o) -> p bb o", bb=GRP),
                    )
                for bi in range(GRP):
                    bb = GRP * q + bi
                    for e in range(2):
                        rhs = wpt_g16.rearrange(
                            "p (bb t o e) -> p bb t o e", t=GRP, o=128, e=2
                        )[:, bb, :, :, e]
                        nc.tensor.matmul(
                            py[:],
                            xt_sb[:, B * (2 * bb + e) : B * (2 * bb + e + 1)],
                            rhs,
                            start=(mm == 0),
                            stop=False,
                        )
                        mm += 1
            # += bias via the delta k-tile (K=128 like every other matmul)
            nc.tensor.matmul(
                py[:],
                xt_sb[:, B * KT : B * (KT + 1)],
                wpt_x[:, 512 * g : 512 * (g + 1)],
                start=False,
                stop=True,
            )
            nc.scalar.copy(y_sb[:, 512 * g : 512 * (g + 1)], py[:])
            nc.sync.dma_start(
                y[:, 512 * g : 512 * (g + 1)],
                y_sb[:, 512 * g : 512 * (g + 1)],
            )

    _strip_self_waits(nc, mybir)
    _split_multi_waits(nc, mybir)
    return nc


def _split_multi_waits(nc, mybir):
    """Hoist all-but-one sync wait from any multi-wait instruction onto
    injected same-engine NoOps placed directly before it.  In-order engines
    (PE/DVE/ACT/SP) execute their queue sequentially, so "nop waits A; inst
    waits B" is equivalent to "inst waits A and B".  Pool is skipped: its 8
    Q7 cores run concurrently and only explicit self-sem waits order them."""
    fn = nc.m.functions[0]
    n = 0
    for b in fn.blocks:
        out = []
        for inst in b.instructions:
            si = inst.sync_info
            eng = str(inst.engine).split(".")[-1]
            if si is not None and len(si.on_wait) >= 2:
                if eng == "Pool":
                    # keep the (load-bearing) Pool self-wait on the
                    # instruction; hoist external waits onto an
                    # EventSemaphore, which blocks the Pool sequencer.
                    keep = [w for w in si.on_wait if w.ant_name.startswith("Pool")]
                    hoist = [w for w in si.on_wait if not w.ant_name.startswith("Pool")]
                    if len(keep) > 1 or not hoist:
                        out.append(inst)
                        continue
                    for w in hoist:
                        nop = mybir.InstEventSemaphore(
                            name=f"ws_psplit_{n}", ins=[], outs=[]
                        )
                        n += 1
                        nop.engine = inst.engine
                        nop.bass_nofuse = True
                        nop.sync_info = mybir.SyncInfo(on_wait=[w], on_update=[])
                        out.append(nop)
                    inst.sync_info = mybir.SyncInfo(
                        on_wait=keep or hoist[-1:], on_update=si.on_update
                    )
                else:
                    for w in si.on_wait[:-1]:
                        nop = mybir.InstNoOp(
                            name=f"ws_split_{n}", ins=[], outs=[]
                        )
                        n += 1
                        nop.engine = inst.engine
                        nop.bass_nofuse = True
                        nop.sync_info = mybir.SyncInfo(on_wait=[w], on_update=[])
                        out.append(nop)
                    inst.sync_info = mybir.SyncInfo(
                        on_wait=si.on_wait[-1:], on_update=si.on_update
                    )
            out.append(inst)
        b.instructions = out


# NOTE: Pool (GPSIMD) is deliberately absent — it is 8 parallel Q7 cores, so
# same-engine ordering does NOT hold there and its self-waits are load-bearing.
_ENGINE_SEM_PREFIX = {
    "PE": "PE_",
    "DVE": "DVE_",
    "Activation": "Activation_",
    "SP": "SP_",
}


def _strip_self_waits(nc, mybir):
    """Several TRN2 ISA instruction structs encode at most ONE sync wait
    (walrus: "Too many sync wait commands").  Two classes of Tile-emitted
    waits are redundant and safe to drop from instructions carrying >=2:

    1. Self-engine waits: an engine completes its own instructions in order.
    2. DMAHW waits on the wq streaming loads: the slot's previous DMA was
       fully consumed by the dequant before the slot-release wait tick, so
       the compute-engine wait transitively covers the DMA-WAW ordering
       (Tile's per-proc vector clock does not track transitivity).
    """
    fn = nc.m.functions[0]
    observed: dict = {}
    for b in fn.blocks:
        for inst in b.instructions:
            si = inst.sync_info
            if si is None or not si.on_wait:
                continue
            eng = str(inst.engine)
            if len(si.on_wait) < 2:
                for w in si.on_wait:
                    k = (eng, w.ant_name)
                    observed[k] = max(observed.get(k, 0), w.wait_value)
                continue
            keep = [
                w
                for w in si.on_wait
                if observed.get((eng, w.ant_name), 0) < w.wait_value
            ]
            pref = _ENGINE_SEM_PREFIX.get(str(inst.engine).split(".")[-1])
            if pref is not None:
                keep = [w for w in keep if not w.ant_name.startswith(pref)]
            if len(keep) >= 2 and type(inst).__name__ == "InstDMACopy":
                if any(
                    not w.ant_name.startswith(("DMAHW", "DMASW")) for w in keep
                ):
                    keep = [
                        w
                        for w in keep
                        if not w.ant_name.startswith(("DMAHW", "DMASW"))
                    ]
            for w in keep:
                k = (eng, w.ant_name)
                observed[k] = max(observed.get(k, 0), w.wait_value)
            if len(keep) != len(si.on_wait):
                inst.sync_info = mybir.SyncInfo(
                    on_wait=keep, on_update=si.on_update
                )


def _get_nc():
    if "nc" not in _CACHE:
        _CACHE["nc"] = _build_nc()
    return _CACHE["nc"]


def _make_in_maps(x, w_q, w_scales, b_q, b_scales):
    x2 = np.ascontiguousarray(x.reshape(B, IN), dtype=np.float32)
    xT = x2.T.astype(np.float16)                          # [3072, 64]
    xt = np.zeros((128, (KT + 1) * B), dtype=np.float16)  # [128, 1600]
    # parity-interleaved k-tiles: column (2*bb+e)*B + b <- xT[256*bb+2*p+e, b]
    xt[:, : KT * B] = (
        xT.reshape(NBB, 128, 2, B).transpose(1, 0, 2, 3).reshape(128, KT * B)
    )
    xt[0, KT * B :] = 1.0                                 # bias delta row
    # int32 codes are 8-bit values: ship as pre-centered int16 (lossless;
    # halves the dominant HBM stream the same way x ships as fp16, and folds
    # the "- 128" into the host layout prep so on-device dequant is a single
    # multiply).
    wq_full = (w_q.reshape(OUT, IN) - 128).astype(np.int16)
    ws_full = np.ascontiguousarray(w_scales, dtype=np.float32)  # [12288, 96]
    bq_full = (b_q.reshape(OUT) - 128).astype(np.int16)
    bs_full = np.ascontiguousarray(b_scales, dtype=np.float32)  # [384]
    ident = np.eye(128, dtype=np.float32)

    in_maps = []
    for c in range(NCORES):
        o0, o1 = c * OSH, (c + 1) * OSH
        ws_c = (
            ws_full[o0:o1]
            .reshape(OT, 128, NB)
            .transpose(1, 0, 2)
            .reshape(128, OT * NB)
        )
        in_maps.append(
            {
                "wq": np.ascontiguousarray(wq_full[o0:o1]),
                "ws": np.ascontiguousarray(ws_c),
                "xt": xt,
                "bq": np.ascontiguousarray(bq_full[o0:o1]).reshape(1, OSH),
                "bs": np.ascontiguousarray(
                    bs_full[o0 // BLOCK : o1 // BLOCK]
                ).reshape(1, OSH // BLOCK),
                "ident": ident,
            }
        )
    return in_maps


def run_shards(x, w_q, w_scales, b_q, b_scales, trace=False):
    """Run the SPMD kernel; returns (y_full, BassKernelResults)."""
    from concourse.bass_utils import run_bass_kernel_spmd

    nc = _get_nc()
    in_maps = _make_in_maps(x, w_q, w_scales, b_q, b_scales)
    res = run_bass_kernel_spmd(
        nc, in_maps, core_ids=list(range(NCORES)), trace=trace
    )
    shards = [np.asarray(res.results[c]["y"]) for c in range(NCORES)]
    y = np.concatenate(shards, axis=1).reshape(B, 1, OUT)
    return y, res


def kernel(**inputs):
    y, _ = run_shards(
        inputs["x"],
        inputs["w_q"],
        inputs["w_scales"],
        inputs["b_q"],
        inputs["b_scales"],
        trace=False,
    )
    return y.astype(np.float32)


# revision 9
# speedup vs baseline: 1.1097x; 1.0553x over previous
"""DequantingLinear Trainium2 kernel, v3.

y = x @ W^T + b where W = (w_q - 128) * w_scales (GGML Q8_0-style, block=32),
b = (b_q - 128) * b_scales.

Sharding: column-parallel over out_features across 8 cores (1536 rows of W per
core).  The codes are 8-bit values; the host ships them as int16 (lossless,
like the host-side fp16 cast of x) so the HBM stream is 9.4 MB/core.

Per core, pipelined per 128-row o-tile with the engines balanced so no single
engine exceeds the others (v2 learning: DVE dequant 44 us, ACT evac 42 us and
PE 51 us all serialized behind group-granular dependencies):

  1. wq shard streams in ([128, IN] int16 tiles, first four as singles)
  2. dequant wp = (wq - 128) * scales -> fp16, fused scalar_tensor_tensor:
     DVE for 9 tiles, GPSIMD (Pool) for the first tile of each group (Pool is
     otherwise idle; per-tile assignment keeps each consumer waiting on ONE
     producer semaphore).  The bias dequant chain lives entirely on Pool too.
  3. PE transposes wp *as fp32 pairs*: a [128, 128] f32 transpose moves two
     adjacent-i fp16 values per element, so 12 transposes per o-tile instead
     of 24 (the per-instruction LDWEIGHTS + sem-wait overhead halves), into
     [128, 512] f32 PSUM banks (4 i-blocks of one tile per bank).  Packed
     pairs are safe: |W| < 3 so the composed f32 exponent field can never be
     all-ones (no NaN/Inf), and a denormal high-half implies the whole block
     scale is tiny so flush-to-zero error is negligible.
  4. ACT evacuates each bank as f32 (half the elements of the fp16 view) into
     a per-group [128, 12x4x128] f32 staging tile.
  5. PE accumulates y[64, 512] per group of 4 o-tiles in fp32 PSUM: for each
     i-block bb and parity e, one N=512 fp16 matmul whose rhs reads the fp16
     view of the staging tile with stride-2 APs; xt is host-interleaved to
     match (partition p of block (bb,e) holds x^T row 256*bb + 2p + e).
     One extra delta-row k-tile adds the Pool-dequantized bias.
  6. y group slices [64, 512] DMA out as they finish; the host concatenates.

Two TRN2 toolchain quirks are handled explicitly (see _strip_self_waits and
_patch_drain_split): every ISA instruction encodes at most ONE semaphore
wait, and walrus refuses multi-wait encodings for several instruction
structs ("Too many sync wait commands").  Cheap same-engine "absorber" ops
take the DMA/slot-release waits up front, a post-pass drops provably
redundant waits, and the kernel-tail drain's global-clock waits are
pre-spread across SP nops.
"""

import sys

import numpy as np

for _p in ("/opt/trn_rl_repo", "/root/.axon_site/_ro/trn_rl_repo"):
    if _p not in sys.path:
        sys.path.append(_p)

B = 64          # batch (x is [64, 1, 3072])
IN = 3072       # in_features
OUT = 12288     # out_features
BLOCK = 32      # quant block
NB = IN // BLOCK            # 96 blocks per row
NCORES = 8
OSH = OUT // NCORES         # 1536 out features per core
OT = OSH // 128             # 12 o-tiles of 128 rows per core
GRP = 4                     # o-tiles per matmul group (N = 512)
NG = OT // GRP              # 3 groups
KT = IN // 128              # 24 contraction tiles
NBB = IN // 256             # 12 fp32-pair i-blocks per o-tile
NSING = 4                   # leading wq tiles loaded as singles

# o-tiles whose dequant runs on GPSIMD (Pool) instead of DVE.  First tile of
# each group: its (slower) dequant overlaps the previous group's compute and
# never gates the group's matmul accumulation tail.
POOL_TILES = frozenset((0, 4, 8))

_CACHE: dict = {}


def _patch_drain_split():
    """The TRN2 ISA gives every instruction exactly ONE inline wait slot;
    Tile's kernel-tail drain asks for the whole global clock (~11 sems) on a
    single instruction, which walrus sometimes refuses ("Too many sync wait
    commands").  Pre-spread those waits across one SP nop per semaphore; the
    drain's own waits then elide via the SP engine clock."""
    from concourse import tile as tile_mod

    if getattr(tile_mod.TileContext, "_drain_split_patched", False):
        return
    from concourse.vector_clock import ScopedClock, VectorClock

    orig = tile_mod.TileContext._drain_and_barrier

    def patched(self, tick_clock, wait_clock):
        gvc = tick_clock.global_clock
        n = len(gvc)
        for p in range(n):
            t = gvc[p]
            if t <= 0:
                continue
            vc = VectorClock([0] * n)
            vc.require_at_least(p, t)
            nop = self.nc.sync.nop(hint="drain_wait_split", nofuse=True)
            wait_clock.add_sem_waits(nop.ins, ScopedClock({None: vc}))
        return orig(self, tick_clock, wait_clock)

    tile_mod.TileContext._drain_and_barrier = patched
    tile_mod.TileContext._drain_split_patched = True


def _build_nc():
    import concourse.bass as bass
    import concourse.mybir as mybir
    from concourse.tile import TileContext
    from contextlib import ExitStack

    _patch_drain_split()

    f32 = mybir.dt.float32
    i16 = mybir.dt.int16
    f16 = mybir.dt.float16

    nc = bass.Bass()
    wq = nc.declare_dram_parameter("wq", [OSH, IN], i16, isOutput=False)
    # ws is host-prearranged: ws[p, t*NB+k] = w_scales[t*128+p, k] so the load
    # is one fully contiguous [128, OT*NB] transfer.
    ws = nc.declare_dram_parameter("ws", [128, OT * NB], f32, isOutput=False)
    # xt is host-prearranged and parity-interleaved: for block bb and parity
    # e, column (2*bb+e)*B + b holds x^T[256*bb + 2*p + e, b] at partition p.
    # The last B columns are a delta at row 0 (bias k-tile).
    xt = nc.declare_dram_parameter("xt", [128, (KT + 1) * B], f16, isOutput=False)
    bq = nc.declare_dram_parameter("bq", [1, OSH], i16, isOutput=False)
    bs = nc.declare_dram_parameter("bs", [1, OSH // BLOCK], f32, isOutput=False)
    ident = nc.declare_dram_parameter("ident", [128, 128], f32, isOutput=False)
    y = nc.declare_dram_parameter("y", [B, OSH], f32, isOutput=True)

    with TileContext(nc) as tc, ExitStack() as ctx:
        const = ctx.enter_context(tc.tile_pool(name="const", bufs=1))
        wq1_pool = ctx.enter_context(tc.tile_pool(name="wq1", bufs=NSING))
        wq_pool = ctx.enter_context(tc.tile_pool(name="wq", bufs=3))
        wp_pool = ctx.enter_context(tc.tile_pool(name="wp", bufs=6))
        wg_pool = ctx.enter_context(tc.tile_pool(name="wg", bufs=2))
        ysb_pool = ctx.enter_context(tc.tile_pool(name="ysb", bufs=1))
        pt_pool = ctx.enter_context(tc.tile_pool(name="pt", bufs=4, space="PSUM"))  # [128,512] f32 = 1 bank each
        py_pool = ctx.enter_context(tc.tile_pool(name="py", bufs=2, space="PSUM"))
        scrap_pool = ctx.enter_context(tc.tile_pool(name="scrap", bufs=1, space="PSUM"))

        # --- constants / small inputs (all fully contiguous transfers),
        # ordered so the first dequant's inputs (scales, identity, wq tile 0)
        # land first ---
        s_all = const.tile([128, OT * NB], f32)
        nc.sync.dma_start(s_all[:], ws[:, :])
        id_sb = const.tile([128, 128], f32)
        nc.sync.dma_start(id_sb[:], ident[:, :])
        wq_first = []
        wq_s = wq1_pool.tile([128, IN], i16)
        nc.sync.dma_start(wq_s[:], wq[0:128, :])
        wq_first.append(wq_s)
        xt_sb = const.tile([128, (KT + 1) * B], f16)
        nc.sync.dma_start(xt_sb[:], xt[:, :])
        for t in range(1, NSING):
            wq_s = wq1_pool.tile([128, IN], i16)
            nc.sync.dma_start(wq_s[:], wq[128 * t : 128 * (t + 1), :])
            wq_first.append(wq_s)
        bq_sb = const.tile([1, OSH], i16)
        nc.sync.dma_start(bq_sb[:], bq[:, :])
        bs_sb = const.tile([1, OSH // BLOCK], f32)
        nc.sync.dma_start(bs_sb[:], bs[:, :])
        wq_pair = []
        for h in range(NSING // 2, OT // 2):
            wq_t = wq_pool.tile([128, 2 * IN], i16)
            nc.sync.dma_start(
                wq_t[:].rearrange("p (t f) -> p t f", t=2),
                wq[256 * h : 256 * (h + 1), :].rearrange(
                    "(t p) f -> p t f", p=128
                ),
            )
            wq_pair.append(wq_t)

        # bias chain on DVE during the ramp (DVE is idle until wq tile 0
        # lands): dequant bias, build the bias row tile (row 0 = bias,
        # rows 1..127 = 0) contracted against the delta k-tile of xt.
        # Single producer semaphore for the bias matmul.
        bias_sb = const.tile([1, OSH], f16)
        nc.vector.scalar_tensor_tensor(
            bias_sb[:].rearrange("o (k j) -> o k j", j=BLOCK),
            bq_sb[:].rearrange("o (k j) -> o k j", j=BLOCK),
            0.0,
            bs_sb[:].unsqueeze(2).broadcast_to([1, OSH // BLOCK, BLOCK]),
            mybir.AluOpType.subtract,
            mybir.AluOpType.mult,
        )
        wpt_x = const.tile([128, OSH], f16)
        nc.vector.memset(wpt_x[:], 0.0)
        nc.vector.tensor_copy(wpt_x[0:1, :], bias_sb[0:1, :])

        y_sb = ysb_pool.tile([B, OSH], f32)
        scrap = scrap_pool.tile([1, 4], f32)  # keeps the psum bank layout

        wp = [None] * OT

        def dequant(t):
            if t < NSING:
                wq_t = wq_first[t][:, :]
            else:
                wq_t = wq_pair[t // 2 - NSING // 2][
                    :, IN * (t % 2) : IN * (t % 2 + 1)
                ]
            wp_t = wp_pool.tile([128, IN], f16)
            wp[t] = wp_t
            # Codes are host-pre-centered (q - 128).  DVE uses the fused STT
            # form with a zero offset — measured 1.7x faster than DVE
            # TensorTensor for the same multiply (1737 vs 2927 ns per half).
            # GPSIMD does not implement STT, so Pool tiles use TensorTensor.
            for hh in range(2):
                sl = slice(hh * IN // 2, (hh + 1) * IN // 2)
                s_bc = (
                    s_all[:, t * NB + hh * NB // 2 : t * NB + (hh + 1) * NB // 2]
                    .unsqueeze(2)
                    .broadcast_to([128, NB // 2, BLOCK])
                )
                if t in POOL_TILES:
                    nc.gpsimd.tensor_tensor(
                        wp_t[:, sl].rearrange("p (k j) -> p k j", j=BLOCK),
                        wq_t[:, sl].rearrange("p (k j) -> p k j", j=BLOCK),
                        s_bc,
                        mybir.AluOpType.mult,
                    )
                else:
                    nc.vector.scalar_tensor_tensor(
                        wp_t[:, sl].rearrange("p (k j) -> p k j", j=BLOCK),
                        wq_t[:, sl].rearrange("p (k j) -> p k j", j=BLOCK),
                        0.0,
                        s_bc,
                        mybir.AluOpType.subtract,
                        mybir.AluOpType.mult,
                    )

        for g in range(NG):
            for t in range(GRP * g, GRP * (g + 1)):
                dequant(t)
            py = py_pool.tile([B, GRP * 128], f32)
            # Per-group staging tile for transposed W: f32 layout
            # [p, (bb, t_loc, o)] — evacs write per (tile, 4-block) bank,
            # matmuls read per (bb, e) with a stride-2 fp16 view.
            wpt_g = wg_pool.tile([128, NBB * GRP * 128], f32)
            wpt_g16 = wpt_g[:].bitcast(f16)
            mm = 0
            for q in range(NBB // GRP):          # 3 pt banks per o-tile
                for j in range(GRP):
                    t = GRP * g + j
                    wp32 = wp[t][:].bitcast(f32)  # [128, 1536]
                    pt = pt_pool.tile([128, 512], f32)
                    for bi in range(GRP):
                        bb = GRP * q + bi
                        nc.tensor.transpose(
                            pt[:, 128 * bi : 128 * (bi + 1)],
                            wp32[:, 128 * bb : 128 * (bb + 1)],
                            id_sb[:],
                        )
                    nc.scalar.copy(
                        wpt_g[:]
                        .rearrange("p (bb t o) -> p bb t o", t=GRP, o=128)[
                            :, GRP * q : GRP * (q + 1), j, :
                        ],
                        pt[:].rearrange("p (bb o) -> p bb o", bb=GRP),
                    )
                for bi in range(GRP):
                    bb = GRP * q + bi
                    for e in range(2):
                        rhs = wpt_g16.rearrange(
                            "p (bb t o e) -> p bb t o e", t=GRP, o=128, e=2
                        )[:, bb, :, :, e]
                        nc.tensor.matmul(
                            py[:],
                            xt_sb[:, B * (2 * bb + e) : B * (2 * bb + e + 1)],
                            rhs,
                            start=(mm == 0),
                            stop=False,
                        )
                        mm += 1
            # += bias via the delta k-tile (K=128 like every other matmul)
            nc.tensor.matmul(
                py[:],
                xt_sb[:, B * KT : B * (KT + 1)],
                wpt_x[:, 512 * g : 512 * (g + 1)],
                start=False,
                stop=True,
            )
            nc.scalar.copy(y_sb[:, 512 * g : 512 * (g + 1)], py[:])
            nc.sync.dma_start(
                y[:, 512 * g : 512 * (g + 1)],
                y_sb[:, 512 * g : 512 * (g + 1)],
            )

    _strip_self_waits(nc, mybir)
    _split_multi_waits(nc, mybir)
    return nc


def _split_multi_waits(nc, mybir):
    """Hoist all-but-one sync wait from any multi-wait instruction onto
    injected same-engine NoOps placed directly before it.  In-order engines
    (PE/DVE/ACT/SP) execute their queue sequentially, so "nop waits A; inst
    waits B" is equivalent to "inst waits A and B".  Pool is skipped: its 8
    Q7 cores run concurrently and only explicit self-sem waits order them."""
    fn = nc.m.functions[0]
    n = 0
    for b in fn.blocks:
        out = []
        for inst in b.instructions:
            si = inst.sync_info
            eng = str(inst.engine).split(".")[-1]
            if si is not None and len(si.on_wait) >= 2:
                if eng == "Pool":
                    # keep the (load-bearing) Pool self-wait on the
                    # instruction; hoist external waits onto an
                    # EventSemaphore, which blocks the Pool sequencer.
                    keep = [w for w in si.on_wait if w.ant_name.startswith("Pool")]
                    hoist = [w for w in si.on_wait if not w.ant_name.startswith("Pool")]
                    if len(keep) > 1 or not hoist:
                        out.append(inst)
                        continue
                    for w in hoist:
                        nop = mybir.InstEventSemaphore(
                            name=f"ws_psplit_{n}", ins=[], outs=[]
                        )
                        n += 1
                        nop.engine = inst.engine
                        nop.bass_nofuse = True
                        nop.sync_info = mybir.SyncInfo(on_wait=[w], on_update=[])
                        out.append(nop)
                    inst.sync_info = mybir.SyncInfo(
                        on_wait=keep or hoist[-1:], on_update=si.on_update
                    )
                else:
                    for w in si.on_wait[:-1]:
                        nop = mybir.InstNoOp(
                            name=f"ws_split_{n}", ins=[], outs=[]
                        )
                        n += 1
                        nop.engine = inst.engine
                        nop.bass_nofuse = True
                        nop.sync_info = mybir.SyncInfo(on_wait=[w], on_update=[])
                        out.append(nop)
                    inst.sync_info = mybir.SyncInfo(
                        on_wait=si.on_wait[-1:], on_update=si.on_update
                    )
            out.append(inst)
        b.instructions = out


# NOTE: Pool (GPSIMD) is deliberately absent — it is 8 parallel Q7 cores, so
# same-engine ordering does NOT hold there and its self-waits are load-bearing.
_ENGINE_SEM_PREFIX = {
    "PE": "PE_",
    "DVE": "DVE_",
    "Activation": "Activation_",
    "SP": "SP_",
}


def _strip_self_waits(nc, mybir):
    """Several TRN2 ISA instruction structs encode at most ONE sync wait
    (walrus: "Too many sync wait commands").  Two classes of Tile-emitted
    waits are redundant and safe to drop from instructions carrying >=2:

    1. Self-engine waits: an engine completes its own instructions in order.
    2. DMAHW waits on the wq streaming loads: the slot's previous DMA was
       fully consumed by the dequant before the slot-release wait tick, so
       the compute-engine wait transitively covers the DMA-WAW ordering
       (Tile's per-proc vector clock does not track transitivity).
    """
    fn = nc.m.functions[0]
    observed: dict = {}
    for b in fn.blocks:
        for inst in b.instructions:
            si = inst.sync_info
            if si is None or not si.on_wait:
                continue
            eng = str(inst.engine)
            if len(si.on_wait) < 2:
                for w in si.on_wait:
                    k = (eng, w.ant_name)
                    observed[k] = max(observed.get(k, 0), w.wait_value)
                continue
            keep = [
                w
                for w in si.on_wait
                if observed.get((eng, w.ant_name), 0) < w.wait_value
            ]
            pref = _ENGINE_SEM_PREFIX.get(str(inst.engine).split(".")[-1])
            if pref is not None:
                keep = [w for w in keep if not w.ant_name.startswith(pref)]
            if len(keep) >= 2 and type(inst).__name__ == "InstDMACopy":
                if any(
                    not w.ant_name.startswith(("DMAHW", "DMASW")) for w in keep
                ):
                    keep = [
                        w
                        for w in keep
                        if not w.ant_name.startswith(("DMAHW", "DMASW"))
                    ]
            for w in keep:
                k = (eng, w.ant_name)
                observed[k] = max(observed.get(k, 0), w.wait_value)
            if len(keep) != len(si.on_wait):
                inst.sync_info = mybir.SyncInfo(
                    on_wait=keep, on_update=si.on_update
                )


def _get_nc():
    if "nc" not in _CACHE:
        _CACHE["nc"] = _build_nc()
    return _CACHE["nc"]


def _make_in_maps(x, w_q, w_scales, b_q, b_scales):
    x2 = np.ascontiguousarray(x.reshape(B, IN), dtype=np.float32)
    xT = x2.T.astype(np.float16)                          # [3072, 64]
    xt = np.zeros((128, (KT + 1) * B), dtype=np.float16)  # [128, 1600]
    # parity-interleaved k-tiles: column (2*bb+e)*B + b <- xT[256*bb+2*p+e, b]
    xt[:, : KT * B] = (
        xT.reshape(NBB, 128, 2, B).transpose(1, 0, 2, 3).reshape(128, KT * B)
    )
    xt[0, KT * B :] = 1.0                                 # bias delta row
    # int32 codes are 8-bit values: ship as pre-centered int16 (lossless;
    # halves the dominant HBM stream the same way x ships as fp16, and folds
    # the "- 128" into the host layout prep so on-device dequant is a single
    # multiply).
    wq_full = (w_q.reshape(OUT, IN) - 128).astype(np.int16)
    ws_full = np.ascontiguousarray(w_scales, dtype=np.float32)  # [12288, 96]
    bq_full = (b_q.reshape(OUT) - 128).astype(np.int16)
    bs_full = np.ascontiguousarray(b_scales, dtype=np.float32)  # [384]
    ident = np.eye(128, dtype=np.float32)

    in_maps = []
    for c in range(NCORES):
        o0, o1 = c * OSH, (c + 1) * OSH
        ws_c = (
            ws_full[o0:o1]
            .reshape(OT, 128, NB)
            .transpose(1, 0, 2)
            .reshape(128, OT * NB)
        )
        in_maps.append(
            {
                "wq": np.ascontiguousarray(wq_full[o0:o1]),
                "ws": np.ascontiguousarray(ws_c),
                "xt": xt,
                "bq": np.ascontiguousarray(bq_full[o0:o1]).reshape(1, OSH),
                "bs": np.ascontiguousarray(
                    bs_full[o0 // BLOCK : o1 // BLOCK]
                ).reshape(1, OSH // BLOCK),
                "ident": ident,
            }
        )
    return in_maps


def run_shards(x, w_q, w_scales, b_q, b_scales, trace=False):
    """Run the SPMD kernel; returns (y_full, BassKernelResults)."""
    from concourse.bass_utils import run_bass_kernel_spmd

    nc = _get_nc()
    in_maps = _make_in_maps(x, w_q, w_scales, b_q, b_scales)
    res = run_bass_kernel_spmd(
        nc, in_maps, core_ids=list(range(NCORES)), trace=trace
    )
    shards = [np.asarray(res.results[c]["y"]) for c in range(NCORES)]
    y = np.concatenate(shards, axis=1).reshape(B, 1, OUT)
    return y, res


def kernel(**inputs):
    y, _ = run_shards(
        inputs["x"],
        inputs["w_q"],
        inputs["w_scales"],
        inputs["b_q"],
        inputs["b_scales"],
        trace=False,
    )
    return y.astype(np.float32)


# revision 10
# speedup vs baseline: 1.2558x; 1.1316x over previous
"""DequantingLinear Trainium2 kernel, v3.

y = x @ W^T + b where W = (w_q - 128) * w_scales (GGML Q8_0-style, block=32),
b = (b_q - 128) * b_scales.

Sharding: column-parallel over out_features across 8 cores (1536 rows of W per
core).  The codes are 8-bit values; the host ships them as int16 (lossless,
like the host-side fp16 cast of x) so the HBM stream is 9.4 MB/core.

Per core, pipelined per 128-row o-tile with the engines balanced so no single
engine exceeds the others (v2 learning: DVE dequant 44 us, ACT evac 42 us and
PE 51 us all serialized behind group-granular dependencies):

  1. wq shard streams in ([128, IN] int16 tiles, first four as singles)
  2. dequant wp = (wq - 128) * scales -> fp16, fused scalar_tensor_tensor:
     DVE for 9 tiles, GPSIMD (Pool) for the first tile of each group (Pool is
     otherwise idle; per-tile assignment keeps each consumer waiting on ONE
     producer semaphore).  The bias dequant chain lives entirely on Pool too.
  3. PE transposes wp *as fp32 pairs*: a [128, 128] f32 transpose moves two
     adjacent-i fp16 values per element, so 12 transposes per o-tile instead
     of 24 (the per-instruction LDWEIGHTS + sem-wait overhead halves), into
     [128, 512] f32 PSUM banks (4 i-blocks of one tile per bank).  Packed
     pairs are safe: |W| < 3 so the composed f32 exponent field can never be
     all-ones (no NaN/Inf), and a denormal high-half implies the whole block
     scale is tiny so flush-to-zero error is negligible.
  4. ACT evacuates each bank as f32 (half the elements of the fp16 view) into
     a per-group [128, 12x4x128] f32 staging tile.
  5. PE accumulates y[64, 512] per group of 4 o-tiles in fp32 PSUM: for each
     i-block bb and parity e, one N=512 fp16 matmul whose rhs reads the fp16
     view of the staging tile with stride-2 APs; xt is host-interleaved to
     match (partition p of block (bb,e) holds x^T row 256*bb + 2p + e).
     One extra delta-row k-tile adds the Pool-dequantized bias.
  6. y group slices [64, 512] DMA out as they finish; the host concatenates.

Two TRN2 toolchain quirks are handled explicitly (see _strip_self_waits and
_patch_drain_split): every ISA instruction encodes at most ONE semaphore
wait, and walrus refuses multi-wait encodings for several instruction
structs ("Too many sync wait commands").  Cheap same-engine "absorber" ops
take the DMA/slot-release waits up front, a post-pass drops provably
redundant waits, and the kernel-tail drain's global-clock waits are
pre-spread across SP nops.
"""

import sys

import numpy as np

for _p in ("/opt/trn_rl_repo", "/root/.axon_site/_ro/trn_rl_repo"):
    if _p not in sys.path:
        sys.path.append(_p)

B = 64          # batch (x is [64, 1, 3072])
IN = 3072       # in_features
OUT = 12288     # out_features
BLOCK = 32      # quant block
NB = IN // BLOCK            # 96 blocks per row
NCORES = 8
OSH = OUT // NCORES         # 1536 out features per core
OT = OSH // 128             # 12 o-tiles of 128 rows per core
GRP = 4                     # o-tiles per matmul group (N = 512)
NG = OT // GRP              # 3 groups
KT = IN // 128              # 24 contraction tiles
NBB = IN // 256             # 12 fp32-pair i-blocks per o-tile
NSING = 4                   # leading wq tiles loaded as singles

# o-tiles whose dequant runs on GPSIMD (Pool) instead of DVE.  Measured: Pool
# TensorTensor is ~3x slower than DVE STT per element AND its SBUF traffic
# slows concurrent DVE STTs by ~35% (Q7 descriptor/SBUF port contention), so
# offloading dequant to Pool is a net loss — keep this empty.
POOL_TILES = frozenset()

_CACHE: dict = {}


def _patch_drain_split():
    """The TRN2 ISA gives every instruction exactly ONE inline wait slot;
    Tile's kernel-tail drain asks for the whole global clock (~11 sems) on a
    single instruction, which walrus sometimes refuses ("Too many sync wait
    commands").  Pre-spread those waits across one SP nop per semaphore; the
    drain's own waits then elide via the SP engine clock."""
    from concourse import tile as tile_mod

    if getattr(tile_mod.TileContext, "_drain_split_patched", False):
        return
    from concourse.vector_clock import ScopedClock, VectorClock

    orig = tile_mod.TileContext._drain_and_barrier

    def patched(self, tick_clock, wait_clock):
        gvc = tick_clock.global_clock
        n = len(gvc)
        for p in range(n):
            t = gvc[p]
            if t <= 0:
                continue
            vc = VectorClock([0] * n)
            vc.require_at_least(p, t)
            nop = self.nc.sync.nop(hint="drain_wait_split", nofuse=True)
            wait_clock.add_sem_waits(nop.ins, ScopedClock({None: vc}))
        return orig(self, tick_clock, wait_clock)

    tile_mod.TileContext._drain_and_barrier = patched
    tile_mod.TileContext._drain_split_patched = True


def _build_nc():
    import concourse.bass as bass
    import concourse.mybir as mybir
    from concourse.tile import TileContext
    from contextlib import ExitStack

    _patch_drain_split()

    f32 = mybir.dt.float32
    i16 = mybir.dt.int16
    f16 = mybir.dt.float16

    nc = bass.Bass()
    wq = nc.declare_dram_parameter("wq", [OSH, IN], i16, isOutput=False)
    # ws is host-prearranged: ws[p, t*NB+k] = w_scales[t*128+p, k] so the load
    # is one fully contiguous [128, OT*NB] transfer.
    ws = nc.declare_dram_parameter("ws", [128, OT * NB], f32, isOutput=False)
    # xt is host-prearranged and parity-interleaved: for block bb and parity
    # e, column (2*bb+e)*B + b holds x^T[256*bb + 2*p + e, b] at partition p.
    # The last B columns are a delta at row 0 (bias k-tile).
    xt = nc.declare_dram_parameter("xt", [128, (KT + 1) * B], f16, isOutput=False)
    bq = nc.declare_dram_parameter("bq", [1, OSH], i16, isOutput=False)
    bs = nc.declare_dram_parameter("bs", [1, OSH // BLOCK], f32, isOutput=False)
    ident = nc.declare_dram_parameter("ident", [128, 128], f32, isOutput=False)
    y = nc.declare_dram_parameter("y", [B, OSH], f32, isOutput=True)

    with TileContext(nc) as tc, ExitStack() as ctx:
        const = ctx.enter_context(tc.tile_pool(name="const", bufs=1))
        wq1_pool = ctx.enter_context(tc.tile_pool(name="wq1", bufs=NSING))
        wq_pool = ctx.enter_context(tc.tile_pool(name="wq", bufs=3))
        wp_pool = ctx.enter_context(tc.tile_pool(name="wp", bufs=6))
        wg_pool = ctx.enter_context(tc.tile_pool(name="wg", bufs=2))
        ysb_pool = ctx.enter_context(tc.tile_pool(name="ysb", bufs=1))
        pt_pool = ctx.enter_context(tc.tile_pool(name="pt", bufs=4, space="PSUM"))  # [128,512] f32 = 1 bank each
        py_pool = ctx.enter_context(tc.tile_pool(name="py", bufs=2, space="PSUM"))
        scrap_pool = ctx.enter_context(tc.tile_pool(name="scrap", bufs=1, space="PSUM"))

        # --- constants / small inputs (all fully contiguous transfers),
        # ordered so the first dequant's inputs (scales, identity, wq tile 0)
        # land first ---
        s_all = const.tile([128, OT * NB], f32)
        nc.sync.dma_start(s_all[:], ws[:, :])
        id_sb = const.tile([128, 128], f32)
        nc.sync.dma_start(id_sb[:], ident[:, :])
        wq_first = []
        wq_s = wq1_pool.tile([128, IN], i16)
        nc.sync.dma_start(wq_s[:], wq[0:128, :])
        wq_first.append(wq_s)
        xt_sb = const.tile([128, (KT + 1) * B], f16)
        nc.sync.dma_start(xt_sb[:], xt[:, :])
        for t in range(1, NSING):
            wq_s = wq1_pool.tile([128, IN], i16)
            nc.sync.dma_start(wq_s[:], wq[128 * t : 128 * (t + 1), :])
            wq_first.append(wq_s)
        bq_sb = const.tile([1, OSH], i16)
        nc.sync.dma_start(bq_sb[:], bq[:, :])
        bs_sb = const.tile([1, OSH // BLOCK], f32)
        nc.sync.dma_start(bs_sb[:], bs[:, :])
        wq_pair = []
        for h in range(NSING // 2, OT // 2):
            wq_t = wq_pool.tile([128, 2 * IN], i16)
            nc.sync.dma_start(
                wq_t[:].rearrange("p (t f) -> p t f", t=2),
                wq[256 * h : 256 * (h + 1), :].rearrange(
                    "(t p) f -> p t f", p=128
                ),
            )
            wq_pair.append(wq_t)

        # bias chain on DVE during the ramp (DVE is idle until wq tile 0
        # lands): dequant bias, build the bias row tile (row 0 = bias,
        # rows 1..127 = 0) contracted against the delta k-tile of xt.
        # Single producer semaphore for the bias matmul.
        bias_sb = const.tile([1, OSH], f16)
        nc.vector.scalar_tensor_tensor(
            bias_sb[:].rearrange("o (k j) -> o k j", j=BLOCK),
            bq_sb[:].rearrange("o (k j) -> o k j", j=BLOCK),
            0.0,
            bs_sb[:].unsqueeze(2).broadcast_to([1, OSH // BLOCK, BLOCK]),
            mybir.AluOpType.subtract,
            mybir.AluOpType.mult,
        )
        wpt_x = const.tile([128, OSH], f16)
        nc.vector.memset(wpt_x[:], 0.0)
        nc.vector.tensor_copy(wpt_x[0:1, :], bias_sb[0:1, :])

        y_sb = ysb_pool.tile([B, OSH], f32)
        scrap = scrap_pool.tile([1, 4], f32)  # keeps the psum bank layout

        wp = [None] * OT

        def dequant(t):
            if t < NSING:
                wq_t = wq_first[t][:, :]
            else:
                wq_t = wq_pair[t // 2 - NSING // 2][
                    :, IN * (t % 2) : IN * (t % 2 + 1)
                ]
            wp_t = wp_pool.tile([128, IN], f16)
            wp[t] = wp_t
            # Codes are host-pre-centered (q - 128).  DVE uses the fused STT
            # form with a zero offset — measured 1.7x faster than DVE
            # TensorTensor for the same multiply (1737 vs 2927 ns per half).
            # GPSIMD does not implement STT, so Pool tiles use TensorTensor.
            for hh in range(2):
                sl = slice(hh * IN // 2, (hh + 1) * IN // 2)
                s_bc = (
                    s_all[:, t * NB + hh * NB // 2 : t * NB + (hh + 1) * NB // 2]
                    .unsqueeze(2)
                    .broadcast_to([128, NB // 2, BLOCK])
                )
                if t in POOL_TILES:
                    nc.gpsimd.tensor_tensor(
                        wp_t[:, sl].rearrange("p (k j) -> p k j", j=BLOCK),
                        wq_t[:, sl].rearrange("p (k j) -> p k j", j=BLOCK),
                        s_bc,
                        mybir.AluOpType.mult,
                    )
                else:
                    nc.vector.scalar_tensor_tensor(
                        wp_t[:, sl].rearrange("p (k j) -> p k j", j=BLOCK),
                        wq_t[:, sl].rearrange("p (k j) -> p k j", j=BLOCK),
                        0.0,
                        s_bc,
                        mybir.AluOpType.subtract,
                        mybir.AluOpType.mult,
                    )

        for g in range(NG):
            for t in range(GRP * g, GRP * (g + 1)):
                dequant(t)
            py = py_pool.tile([B, GRP * 128], f32)
            # Per-group staging tile for transposed W: f32 layout
            # [p, (bb, t_loc, o)] — evacs write per (tile, 4-block) bank,
            # matmuls read per (bb, e) with a stride-2 fp16 view.
            wpt_g = wg_pool.tile([128, NBB * GRP * 128], f32)
            wpt_g16 = wpt_g[:].bitcast(f16)
            mm = 0
            for q in range(NBB // GRP):          # 3 pt banks per o-tile
                for j in range(GRP):
                    t = GRP * g + j
                    wp32 = wp[t][:].bitcast(f32)  # [128, 1536]
                    pt = pt_pool.tile([128, 512], f32)
                    for bi in range(GRP):
                        bb = GRP * q + bi
                        nc.tensor.transpose(
                            pt[:, 128 * bi : 128 * (bi + 1)],
                            wp32[:, 128 * bb : 128 * (bb + 1)],
                            id_sb[:],
                        )
                    nc.scalar.copy(
                        wpt_g[:]
                        .rearrange("p (bb t o) -> p bb t o", t=GRP, o=128)[
                            :, GRP * q : GRP * (q + 1), j, :
                        ],
                        pt[:].rearrange("p (bb o) -> p bb o", bb=GRP),
                    )
                for bi in range(GRP):
                    bb = GRP * q + bi
                    for e in range(2):
                        rhs = wpt_g16.rearrange(
                            "p (bb t o e) -> p bb t o e", t=GRP, o=128, e=2
                        )[:, bb, :, :, e]
                        nc.tensor.matmul(
                            py[:],
                            xt_sb[:, B * (2 * bb + e) : B * (2 * bb + e + 1)],
                            rhs,
                            start=(mm == 0),
                            stop=False,
                        )
                        mm += 1
            # += bias via the delta k-tile (K=128 like every other matmul)
            nc.tensor.matmul(
                py[:],
                xt_sb[:, B * KT : B * (KT + 1)],
                wpt_x[:, 512 * g : 512 * (g + 1)],
                start=False,
                stop=True,
            )
            nc.scalar.copy(y_sb[:, 512 * g : 512 * (g + 1)], py[:])
            nc.sync.dma_start(
                y[:, 512 * g : 512 * (g + 1)],
                y_sb[:, 512 * g : 512 * (g + 1)],
            )

    _strip_self_waits(nc, mybir)
    _split_multi_waits(nc, mybir)
    return nc


def _split_multi_waits(nc, mybir):
    """Hoist all-but-one sync wait from any multi-wait instruction onto
    injected same-engine NoOps placed directly before it.  In-order engines
    (PE/DVE/ACT/SP) execute their queue sequentially, so "nop waits A; inst
    waits B" is equivalent to "inst waits A and B".  Pool is skipped: its 8
    Q7 cores run concurrently and only explicit self-sem waits order them."""
    fn = nc.m.functions[0]
    n = 0
    for b in fn.blocks:
        out = []
        for inst in b.instructions:
            si = inst.sync_info
            eng = str(inst.engine).split(".")[-1]
            if si is not None and len(si.on_wait) >= 2:
                if eng == "Pool":
                    # keep the (load-bearing) Pool self-wait on the
                    # instruction; hoist external waits onto an
                    # EventSemaphore, which blocks the Pool sequencer.
                    keep = [w for w in si.on_wait if w.ant_name.startswith("Pool")]
                    hoist = [w for w in si.on_wait if not w.ant_name.startswith("Pool")]
                    if len(keep) > 1 or not hoist:
                        out.append(inst)
                        continue
                    for w in hoist:
                        nop = mybir.InstEventSemaphore(
                            name=f"ws_psplit_{n}", ins=[], outs=[]
                        )
                        n += 1
                        nop.engine = inst.engine
                        nop.bass_nofuse = True
                        nop.sync_info = mybir.SyncInfo(on_wait=[w], on_update=[])
                        out.append(nop)
                    inst.sync_info = mybir.SyncInfo(
                        on_wait=keep or hoist[-1:], on_update=si.on_update
                    )
                else:
                    # EventSemaphore, not NoOp: a NoOp lowers to DRAIN, which
                    # quiesces the engine pipeline (~570 ns on DVE); an
                    # EventSemaphore wait costs ~130 ns.
                    for w in si.on_wait[:-1]:
                        nop = mybir.InstEventSemaphore(
                            name=f"ws_split_{n}", ins=[], outs=[]
                        )
                        n += 1
                        nop.engine = inst.engine
                        nop.bass_nofuse = True
                        nop.sync_info = mybir.SyncInfo(on_wait=[w], on_update=[])
                        out.append(nop)
                    inst.sync_info = mybir.SyncInfo(
                        on_wait=si.on_wait[-1:], on_update=si.on_update
                    )
            out.append(inst)
        b.instructions = out


# NOTE: Pool (GPSIMD) is deliberately absent — it is 8 parallel Q7 cores, so
# same-engine ordering does NOT hold there and its self-waits are load-bearing.
_ENGINE_SEM_PREFIX = {
    "PE": "PE_",
    "DVE": "DVE_",
    "Activation": "Activation_",
    "SP": "SP_",
}


def _strip_self_waits(nc, mybir):
    """Several TRN2 ISA instruction structs encode at most ONE sync wait
    (walrus: "Too many sync wait commands").  Two classes of Tile-emitted
    waits are redundant and safe to drop from instructions carrying >=2:

    1. Self-engine waits: an engine completes its own instructions in order.
    2. DMAHW waits on the wq streaming loads: the slot's previous DMA was
       fully consumed by the dequant before the slot-release wait tick, so
       the compute-engine wait transitively covers the DMA-WAW ordering
       (Tile's per-proc vector clock does not track transitivity).
    """
    fn = nc.m.functions[0]
    observed: dict = {}
    for b in fn.blocks:
        for inst in b.instructions:
            si = inst.sync_info
            if si is None or not si.on_wait:
                continue
            eng = str(inst.engine)
            if len(si.on_wait) < 2:
                for w in si.on_wait:
                    k = (eng, w.ant_name)
                    observed[k] = max(observed.get(k, 0), w.wait_value)
                continue
            keep = [
                w
                for w in si.on_wait
                if observed.get((eng, w.ant_name), 0) < w.wait_value
            ]
            pref = _ENGINE_SEM_PREFIX.get(str(inst.engine).split(".")[-1])
            if pref is not None:
                keep = [w for w in keep if not w.ant_name.startswith(pref)]
            if len(keep) >= 2 and type(inst).__name__ == "InstDMACopy":
                if any(
                    not w.ant_name.startswith(("DMAHW", "DMASW")) for w in keep
                ):
                    keep = [
                        w
                        for w in keep
                        if not w.ant_name.startswith(("DMAHW", "DMASW"))
                    ]
            for w in keep:
                k = (eng, w.ant_name)
                observed[k] = max(observed.get(k, 0), w.wait_value)
            if len(keep) != len(si.on_wait):
                inst.sync_info = mybir.SyncInfo(
                    on_wait=keep, on_update=si.on_update
                )


def _get_nc():
    if "nc" not in _CACHE:
        _CACHE["nc"] = _build_nc()
    return _CACHE["nc"]


def _make_in_maps(x, w_q, w_scales, b_q, b_scales):
    x2 = np.ascontiguousarray(x.reshape(B, IN), dtype=np.float32)
    xT = x2.T.astype(np.float16)                          # [3072, 64]
    xt = np.zeros((128, (KT + 1) * B), dtype=np.float16)  # [128, 1600]
    # parity-interleaved k-tiles: column (2*bb+e)*B + b <- xT[256*bb+2*p+e, b]
    xt[:, : KT * B] = (
        xT.reshape(NBB, 128, 2, B).transpose(1, 0, 2, 3).reshape(128, KT * B)
    )
    xt[0, KT * B :] = 1.0                                 # bias delta row
    # int32 codes are 8-bit values: ship as pre-centered int16 (lossless;
    # halves the dominant HBM stream the same way x ships as fp16, and folds
    # the "- 128" into the host layout prep so on-device dequant is a single
    # multiply).
    wq_full = (w_q.reshape(OUT, IN) - 128).astype(np.int16)
    ws_full = np.ascontiguousarray(w_scales, dtype=np.float32)  # [12288, 96]
    bq_full = (b_q.reshape(OUT) - 128).astype(np.int16)
    bs_full = np.ascontiguousarray(b_scales, dtype=np.float32)  # [384]
    ident = np.eye(128, dtype=np.float32)

    in_maps = []
    for c in range(NCORES):
        o0, o1 = c * OSH, (c + 1) * OSH
        ws_c = (
            ws_full[o0:o1]
            .reshape(OT, 128, NB)
            .transpose(1, 0, 2)
            .reshape(128, OT * NB)
        )
        in_maps.append(
            {
                "wq": np.ascontiguousarray(wq_full[o0:o1]),
                "ws": np.ascontiguousarray(ws_c),
                "xt": xt,
                "bq": np.ascontiguousarray(bq_full[o0:o1]).reshape(1, OSH),
                "bs": np.ascontiguousarray(
                    bs_full[o0 // BLOCK : o1 // BLOCK]
                ).reshape(1, OSH // BLOCK),
                "ident": ident,
            }
        )
    return in_maps


def run_shards(x, w_q, w_scales, b_q, b_scales, trace=False):
    """Run the SPMD kernel; returns (y_full, BassKernelResults)."""
    from concourse.bass_utils import run_bass_kernel_spmd

    nc = _get_nc()
    in_maps = _make_in_maps(x, w_q, w_scales, b_q, b_scales)
    res = run_bass_kernel_spmd(
        nc, in_maps, core_ids=list(range(NCORES)), trace=trace
    )
    shards = [np.asarray(res.results[c]["y"]) for c in range(NCORES)]
    y = np.concatenate(shards, axis=1).reshape(B, 1, OUT)
    return y, res


def kernel(**inputs):
    y, _ = run_shards(
        inputs["x"],
        inputs["w_q"],
        inputs["w_scales"],
        inputs["b_q"],
        inputs["b_scales"],
        trace=False,
    )
    return y.astype(np.float32)
